# revision 4
# baseline (speedup 1.0000x reference)
"""Trainium2 Bass kernel for nn_DEQEQFusionBlock_80642305949812.

DEQ fusion block: reference runs 30 Anderson-accelerated fixed-point
iterations of a conv-gated fusion function plus one final application.
The map is contractive (|J| ~ 0.62), so 31 plain Picard applications
converge to the same fixed point to within the reference's own
convergence error (~5e-6 relative). That removes the Anderson
gram/solve entirely and makes the computation embarrassingly parallel.

Sharding: 8 cores = batch(4) x T-halves(2). Each core iterates on its
T-half extended by a ghost margin that shrinks by 2 columns/side per
application (conv halo), so there is NO inter-core communication.
Right-half cores receive T-reversed features and k-flipped conv weights
so a single SPMD program serves both sides.

Precision: apps 1..23 use float32r matmuls (FP22 multiply path, 1
cycle/row), apps 24..31 native fp32 (4 cycles/row). Residual fp32r
error contracts by 0.62^8 through the fp32 phase.
"""
import numpy as np
from contextlib import ExitStack

import concourse.bass as bass
import concourse.mybir as mybir
import concourse.tile as tile
import concourse.bacc as bacc
from concourse import bass_isa
from concourse.bass_utils import run_bass_kernel_spmd

P = 128
C = 256            # channels per block
B, T, K = 4, 512, 3
A = 31             # total Picard applications (incl. the final one)
NR = 23            # apps 1..NR run in float32r, rest in fp32
HALF = T // 2      # per-core output columns
FW = HALF + 2 * (A - 1) + 2   # feature data cols = 318 (u_1 rounded to even)
EPS = 1e-5

f32 = mybir.dt.float32
f32r = mybir.dt.float32r
AF = mybir.ActivationFunctionType
ALU = mybir.AluOpType

# conv order: index into the stationary weight array
CONVS = ["mb0_Wf", "mb0_Wg", "mb1_Wf", "mb1_Wg",
         "fb_Wgate0", "fb_Wproj0", "fb_Wgate1", "fb_Wproj1", "fb_Wself"]
NTILES = 9 * 3 * 2 * 2  # 108 stationary tiles of [128, 128]


def _w(j):
    # data width of z_j / of_j
    return HALF + 2 * (A - j)


def _prec(j):
    return 'r' if j <= NR else 'f'


def _tidx(cv, k, ci, co):
    return ((cv * 3 + k) * 2 + ci) * 2 + co


def build_nc():
    nc = bacc.Bacc("TRN2", target_bir_lowering=False, num_devices=8)
    f_d = [nc.dram_tensor(f"f{i}", [P, 2, FW + 1], f32, kind="ExternalInput")
           for i in range(3)]
    w_d = nc.dram_tensor("wst", [P, NTILES, P], f32, kind="ExternalInput")
    gb_d = nc.dram_tensor("gb", [P, 2, 6], f32, kind="ExternalInput")
    out_d = nc.dram_tensor("out", [P, 6, HALF], f32, kind="ExternalOutput")

    with tile.TileContext(nc) as tc, ExitStack() as ctx:
        const = ctx.enter_context(tc.tile_pool(name="const", bufs=1))
        po0 = ctx.enter_context(tc.tile_pool(name="po0", bufs=2))
        po1 = ctx.enter_context(tc.tile_pool(name="po1", bufs=2))
        pof = ctx.enter_context(tc.tile_pool(name="pof", bufs=2))
        pdup = ctx.enter_context(tc.tile_pool(name="pdup", bufs=1))
        tmp = ctx.enter_context(tc.tile_pool(name="tmp", bufs=2))
        ser = ctx.enter_context(tc.tile_pool(name="ser", bufs=1))
        ps = ctx.enter_context(tc.tile_pool(name="ps", bufs=8, space="PSUM"))

        # ---- loads ----
        wr = const.tile([P, NTILES, P], f32r)
        wf = const.tile([P, NTILES, P], f32)
        nc.sync.dma_start(out=wr, in_=w_d.ap().bitcast(f32r))
        nc.sync.dma_start(out=wf, in_=w_d.ap())
        feats = []
        for i in range(3):
            ft = const.tile([P, 2, FW + 1], f32, tag=f"feat{i}")
            nc.sync.dma_start(out=ft, in_=f_d[i].ap())
            feats.append(ft)
        gb = const.tile([P, 2, 6], f32)
        nc.sync.dma_start(out=gb, in_=gb_d.ap())
        eps = const.tile([P, 1], f32)
        nc.vector.memset(eps, EPS)
        zc = const.tile([P, 2, 1], f32)
        nc.vector.memset(zc, 0.0)

        def conv(dst_ps, cv, src, co, n, prec):
            """accumulate conv cv, out-chunk co, width n into psum dst_ps.
            src: [P, 2, *] tile; out phys cols [1, 1+n) read src [k, k+n)."""
            W = wr if prec == 'r' else wf
            first = True
            for ci in range(2):
                for k in range(3):
                    nc.tensor.matmul(
                        out=dst_ps[:, 0:n],
                        lhsT=W[:, _tidx(cv, k, ci, co), :],
                        rhs=src[:, ci, k:k + n],
                        start=first, stop=(ci == 1 and k == 2))
                    first = False

        def cln(x, n, gi, o_tile, extra_dtype=None, extra_tile=None):
            """channel layernorm of x [P, 2, n] -> o_tile[:, :, 1:1+n].
            gi: gamma/beta column pair index (gamma=2*gi, beta=2*gi+1).
            If extra_tile is given, also write a copy there (dtype differs)."""
            sq = tmp.tile([P, 2, FW], f32, tag="sq")
            for c in range(2):
                nc.scalar.activation(out=sq[:, c, 0:n], in_=x[:, c, :],
                                     func=AF.Square)
            ar0 = ser.tile([P, FW], f32, tag="ar0")
            ar1 = ser.tile([P, FW], f32, tag="ar1")
            aq0 = ser.tile([P, FW], f32, tag="aq0")
            aq1 = ser.tile([P, FW], f32, tag="aq1")
            nc.gpsimd.partition_all_reduce(ar0[:, 0:n], x[:, 0, :], channels=P,
                                           reduce_op=bass_isa.ReduceOp.add)
            nc.gpsimd.partition_all_reduce(ar1[:, 0:n], x[:, 1, :], channels=P,
                                           reduce_op=bass_isa.ReduceOp.add)
            nc.gpsimd.partition_all_reduce(aq0[:, 0:n], sq[:, 0, 0:n], channels=P,
                                           reduce_op=bass_isa.ReduceOp.add)
            nc.gpsimd.partition_all_reduce(aq1[:, 0:n], sq[:, 1, 0:n], channels=P,
                                           reduce_op=bass_isa.ReduceOp.add)
            s1 = ser.tile([P, FW], f32, tag="s1")
            s2 = ser.tile([P, FW], f32, tag="s2")
            nc.vector.tensor_add(out=s1[:, 0:n], in0=ar0[:, 0:n], in1=ar1[:, 0:n])
            nc.vector.tensor_add(out=s2[:, 0:n], in0=aq0[:, 0:n], in1=aq1[:, 0:n])
            t = ser.tile([P, FW], f32, tag="t")
            nc.vector.tensor_mul(out=t[:, 0:n], in0=s1[:, 0:n], in1=s1[:, 0:n])
            vp = ser.tile([P, FW], f32, tag="vp")
            nc.vector.scalar_tensor_tensor(out=vp[:, 0:n], in0=t[:, 0:n],
                                           scalar=-1.0 / C, in1=s2[:, 0:n],
                                           op0=ALU.mult, op1=ALU.add)
            sd = ser.tile([P, FW], f32, tag="sd")
            nc.scalar.activation(out=sd[:, 0:n], in_=vp[:, 0:n], func=AF.Sqrt,
                                 scale=1.0 / C, bias=eps[:, :])
            rs = ser.tile([P, FW], f32, tag="rs")
            nc.vector.reciprocal(out=rs[:, 0:n], in_=sd[:, 0:n])
            mu = ser.tile([P, FW], f32, tag="mu")
            nc.vector.scalar_tensor_tensor(out=mu[:, 0:n], in0=s1[:, 0:n],
                                           scalar=1.0 / C, in1=rs[:, 0:n],
                                           op0=ALU.mult, op1=ALU.mult)
            for c in range(2):
                t1 = ser.tile([P, FW], f32, tag="t1")
                nc.vector.tensor_mul(out=t1[:, 0:n], in0=x[:, c, :], in1=rs[:, 0:n])
                t2 = ser.tile([P, FW], f32, tag="t2")
                nc.vector.tensor_sub(out=t2[:, 0:n], in0=t1[:, 0:n], in1=mu[:, 0:n])
                nc.scalar.activation(out=o_tile[:, c, 1:1 + n], in_=t2[:, 0:n],
                                     func=AF.Identity,
                                     scale=gb[:, c, 2 * gi:2 * gi + 1],
                                     bias=gb[:, c, 2 * gi + 1:2 * gi + 2])
                if extra_tile is not None:
                    nc.vector.tensor_copy(out=extra_tile[:, c, 1:1 + n],
                                          in_=o_tile[:, c, 1:1 + n].bitcast(f32))

        o_prev = [None, None]   # o0_{j-1}, o1_{j-1} (as read by mb convs)
        of_prev = None
        for j in range(1, A + 1):
            w = _w(j)
            u = w + 2      # o-block compute width, rounded up to even (fp32r
                           # matmuls require an even moving free-dim)
            pj = _prec(j)
            pnext = _prec(j + 1) if j < A else 'f'
            dt_o = f32r if pj == 'r' else f32
            dt_of = f32r if pnext == 'r' else f32
            need_dup = (pj == 'r' and pnext == 'f')

            o_cur = []
            dup_cur = []
            for m in range(2):
                pool_m = po0 if m == 0 else po1
                o_m = pool_m.tile([P, 2, FW + 1], dt_o, tag=f"o{m}")
                nc.vector.tensor_copy(out=o_m[:, :, 0:1], in_=zc)
                dup_m = None
                if need_dup:
                    dup_m = pdup.tile([P, 2, FW + 1], f32, tag=f"dup{m}")
                    nc.vector.tensor_copy(out=dup_m[:, :, 0:1], in_=zc)
                if j == 1:
                    # z_0 = 0: h = 0, o = cln(feat)
                    cln(feats[m][:, :, 1:1 + u], u, m, o_m,
                        extra_tile=dup_m)
                else:
                    xin = tmp.tile([P, 2, FW], f32, tag=f"xin{m}")
                    for co in range(2):
                        pf = ps.tile([P, FW], f32, tag="ps")
                        conv(pf, 2 * m + 0, o_prev[m], co, u, pj)
                        pg = ps.tile([P, FW], f32, tag="ps")
                        conv(pg, 2 * m + 1, o_prev[m], co, u, pj)
                        tf = tmp.tile([P, FW], f32, tag="tf")
                        nc.scalar.activation(out=tf[:, 0:u], in_=pf[:, 0:u],
                                             func=AF.Tanh)
                        tg = tmp.tile([P, FW], f32, tag="tg")
                        nc.scalar.activation(out=tg[:, 0:u], in_=pg[:, 0:u],
                                             func=AF.Sigmoid)
                        hx = tmp.tile([P, FW], f32, tag="hx")
                        nc.vector.tensor_mul(out=hx[:, 0:u], in0=tf[:, 0:u],
                                             in1=tg[:, 0:u])
                        nc.vector.tensor_add(out=xin[:, co, 0:u], in0=hx[:, 0:u],
                                             in1=feats[m][:, co, 1:1 + u])
                    cln(xin[:, :, 0:u], u, m, o_m, extra_tile=dup_m)
                o_cur.append(o_m)
                dup_cur.append(dup_m)

            # fusion block
            acc = tmp.tile([P, 2, FW], f32, tag="acc")
            for m in range(2):
                for co in range(2):
                    pg = ps.tile([P, FW], f32, tag="ps")
                    conv(pg, 4 + 2 * m, o_cur[m], co, w, pj)     # gate
                    pp = ps.tile([P, FW], f32, tag="ps")
                    conv(pp, 5 + 2 * m, o_cur[m], co, w, pj)     # proj
                    sg = tmp.tile([P, FW], f32, tag="sg")
                    nc.scalar.activation(out=sg[:, 0:w], in_=pg[:, 0:w],
                                         func=AF.Sigmoid)
                    if m == 0:
                        nc.vector.tensor_mul(out=acc[:, co, 0:w], in0=sg[:, 0:w],
                                             in1=pp[:, 0:w])
                    else:
                        gp = tmp.tile([P, FW], f32, tag="gp")
                        nc.vector.tensor_mul(out=gp[:, 0:w], in0=sg[:, 0:w],
                                             in1=pp[:, 0:w])
                        nc.vector.tensor_add(out=acc[:, co, 0:w],
                                             in0=acc[:, co, 0:w], in1=gp[:, 0:w])
            xf = tmp.tile([P, 2, FW], f32, tag="xf")
            for co in range(2):
                if j == 1:
                    nc.vector.tensor_add(out=xf[:, co, 0:w], in0=acc[:, co, 0:w],
                                         in1=feats[2][:, co, 1:1 + w])
                else:
                    pslf = ps.tile([P, FW], f32, tag="ps")
                    conv(pslf, 8, of_prev, co, w, pj)
                    nc.vector.tensor_add(out=xf[:, co, 0:w], in0=pslf[:, 0:w],
                                         in1=acc[:, co, 0:w])
                    nc.vector.tensor_add(out=xf[:, co, 0:w], in0=xf[:, co, 0:w],
                                         in1=feats[2][:, co, 1:1 + w])
            of_t = pof.tile([P, 2, FW + 1], dt_of, tag="of")
            nc.vector.tensor_copy(out=of_t[:, :, 0:1], in_=zc)
            cln(xf[:, :, 0:w], w, 2, of_t)

            o_prev = [dup_cur[0] if need_dup else o_cur[0],
                      dup_cur[1] if need_dup else o_cur[1]]
            of_prev = of_t

            if j == A:
                nc.sync.dma_start(out=out_d.ap()[:, 0:2, :],
                                  in_=o_cur[0][:, :, 1:1 + HALF].bitcast(f32))
                nc.sync.dma_start(out=out_d.ap()[:, 2:4, :],
                                  in_=o_cur[1][:, :, 1:1 + HALF].bitcast(f32))
                nc.sync.dma_start(out=out_d.ap()[:, 4:6, :],
                                  in_=of_t[:, :, 1:1 + HALF].bitcast(f32))

    nc.compile()
    return nc


_NC = None


def _get_nc():
    global _NC
    if _NC is None:
        _NC = build_nc()
    return _NC


def _prep_inputs(inputs):
    """Build the 8 per-core input maps from the full problem inputs."""
    feats = [np.asarray(inputs[f"feat{i}"], np.float32) for i in range(3)]
    # stationary weights, one per side (k flipped for the mirrored side)
    wst = []
    for side in range(2):
        arr = np.empty((P, NTILES, P), np.float32)
        for cv, name in enumerate(CONVS):
            Wt = np.asarray(inputs[name], np.float32)   # [co, ci, k]
            if side == 1:
                Wt = Wt[:, :, ::-1]
            # arr[p, tidx(cv,k,ci,co), m] = Wt[co*128+m, ci*128+p, k]
            r = Wt.reshape(2, P, 2, P, 3)                # [coc, m, cic, p, k]
            r = r.transpose(3, 4, 2, 0, 1)               # [p, k, cic, coc, m]
            arr[:, cv * 12:(cv + 1) * 12, :] = r.reshape(P, 12, P)
        wst.append(np.ascontiguousarray(arr))
    gba = np.empty((P, 2, 6), np.float32)
    for gi, (gn, bn) in enumerate([("mb0_gamma", "mb0_beta"),
                                   ("mb1_gamma", "mb1_beta"),
                                   ("fb_gamma", "fb_beta")]):
        gba[:, :, 2 * gi] = np.asarray(inputs[gn], np.float32).reshape(2, P).T
        gba[:, :, 2 * gi + 1] = np.asarray(inputs[bn], np.float32).reshape(2, P).T
    gba = np.ascontiguousarray(gba)

    in_maps = []
    for c in range(8):
        b, side = c // 2, c % 2
        m = {"wst": wst[side], "gb": gba}
        for i in range(3):
            ft = np.zeros((P, 2, FW + 1), np.float32)
            sl = feats[i][b, :, 0:FW] if side == 0 else feats[i][b, :, T - FW:][:, ::-1]
            ft[:, :, 1:] = sl.reshape(2, P, FW).transpose(1, 0, 2)
            m[f"f{i}"] = ft
        in_maps.append(m)
    return in_maps


def run(inputs, **kw):
    nc = _get_nc()
    in_maps = _prep_inputs(inputs)
    res = run_bass_kernel_spmd(nc, in_maps, list(range(8)), **kw)
    out = np.empty((B, 3 * C, T), np.float32)
    for c in range(8):
        b, side = c // 2, c % 2
        o = res.results[c]["out"]            # [P, 6, HALF]
        o = o.transpose(1, 0, 2).reshape(3 * C, HALF)   # rows blk*128+p
        if side == 0:
            out[b, :, 0:HALF] = o
        else:
            out[b, :, HALF:] = o[:, ::-1]
    return out, res


def kernel(**inputs) -> np.ndarray:
    out, _ = run(inputs)
    return out


# revision 10
# speedup vs baseline: 1.2421x; 1.2421x over previous
"""Trainium2 Bass kernel for nn_DEQEQFusionBlock_80642305949812.

DEQ fusion block: reference runs 30 Anderson-accelerated fixed-point
iterations of a conv-gated fusion function plus one final application.
The map is contractive (|J| ~ 0.62), so 31 plain Picard applications
converge to the same fixed point to within the reference's own
convergence error (~5e-6 relative). That removes the Anderson
gram/solve entirely and makes the computation embarrassingly parallel.

Sharding: 8 cores = batch(4) x T-halves(2). Each core iterates on its
T-half extended by a ghost margin that shrinks by 2 columns/side per
application (conv halo), so there is NO inter-core communication.
Right-half cores receive T-reversed features and k-flipped conv weights
so a single SPMD program serves both sides.

Precision: apps 1..23 use float32r matmuls (FP22 multiply path, 1
cycle/row), apps 24..31 native fp32 (4 cycles/row). Residual fp32r
error contracts by 0.62^8 through the fp32 phase.
"""
import numpy as np
from contextlib import ExitStack

import concourse.bass as bass
import concourse.mybir as mybir
import concourse.tile as tile
import concourse.bacc as bacc
from concourse import bass_isa
from concourse.bass_utils import run_bass_kernel_spmd

P = 128
C = 256            # channels per block
B, T, K = 4, 512, 3
A = 31             # total Picard applications (incl. the final one)
NR = 23            # apps 1..NR run in float32r, rest in fp32
HALF = T // 2      # per-core output columns
FW = HALF + 2 * (A - 1) + 2   # feature data cols = 318 (u_1 rounded to even)
EPS = 1e-5

f32 = mybir.dt.float32
f32r = mybir.dt.float32r
AF = mybir.ActivationFunctionType
ALU = mybir.AluOpType

# conv order: index into the stationary weight array
CONVS = ["mb0_Wf", "mb0_Wg", "mb1_Wf", "mb1_Wg",
         "fb_Wgate0", "fb_Wproj0", "fb_Wgate1", "fb_Wproj1", "fb_Wself"]
NTILES = 9 * 3 * 2 * 2  # 108 stationary tiles of [128, 128]


def _w(j):
    # data width of z_j / of_j
    return HALF + 2 * (A - j)


def _prec(j):
    return 'r' if j <= NR else 'f'


def _tidx(cv, k, ci, co):
    return ((cv * 3 + k) * 2 + ci) * 2 + co


def build_nc(repeat=1):
    nc = bacc.Bacc("TRN2", target_bir_lowering=False, num_devices=8)
    f_d = [nc.dram_tensor(f"f{i}", [P, 2, FW + 1], f32, kind="ExternalInput")
           for i in range(3)]
    w_d = nc.dram_tensor("wst", [P, NTILES, P], f32, kind="ExternalInput")
    gb_d = nc.dram_tensor("gb", [P, 2, 6], f32, kind="ExternalInput")
    out_d = nc.dram_tensor("out", [P, 6, HALF], f32, kind="ExternalOutput")

    with tile.TileContext(nc) as tc, ExitStack() as ctx:
        const = ctx.enter_context(tc.tile_pool(name="const", bufs=1))
        po0 = ctx.enter_context(tc.tile_pool(name="po0", bufs=2))
        po1 = ctx.enter_context(tc.tile_pool(name="po1", bufs=2))
        pof = ctx.enter_context(tc.tile_pool(name="pof", bufs=2))
        pdup = ctx.enter_context(tc.tile_pool(name="pdup", bufs=1))
        tmp = ctx.enter_context(tc.tile_pool(name="tmp", bufs=2))
        ser = ctx.enter_context(tc.tile_pool(name="ser", bufs=2))
        ps = ctx.enter_context(tc.tile_pool(name="ps", bufs=8, space="PSUM"))

        # ---- loads ----
        wr = const.tile([P, NTILES, P], f32r)
        wf = const.tile([P, NTILES, P], f32)
        nc.sync.dma_start(out=wr[:, 48:NTILES, :], in_=w_d.ap()[:, 48:NTILES, :].bitcast(f32r))
        nc.sync.dma_start(out=wr[:, 0:48, :], in_=w_d.ap()[:, 0:48, :].bitcast(f32r))
        nc.sync.dma_start(out=wf, in_=w_d.ap())
        feats = []
        for i in range(3):
            ft = const.tile([P, 2, FW + 1], f32, tag=f"feat{i}")
            nc.sync.dma_start(out=ft, in_=f_d[i].ap())
            feats.append(ft)
        gb = const.tile([P, 2, 6], f32)
        nc.sync.dma_start(out=gb, in_=gb_d.ap())
        eps = const.tile([P, 1], f32)
        nc.vector.memset(eps, EPS)
        zc = const.tile([P, 2, 1], f32)
        nc.vector.memset(zc, 0.0)

        def conv(dst_ps, cv, src, co, lo, hi, prec):
            """accumulate conv cv out-chunk co for logical cols [lo, hi) into
            psum dst_ps[:, 0:hi-lo]. src: [P, 2, *] tile, phys col = t + 1."""
            W = wr if prec == 'r' else wf
            first = True
            for ci in range(2):
                for k in range(3):
                    nc.tensor.matmul(
                        out=dst_ps[:, 0:hi - lo],
                        lhsT=W[:, _tidx(cv, k, ci, co), :],
                        rhs=src[:, ci, lo + k:hi + k],
                        start=first, stop=(ci == 1 and k == 2))
                    first = False

        SW = FW  # single full-width stripe (fp32r needs N>=256)

        def cln(x, lo, hi, gi, o_tile, extra_tile=None):
            """channel layernorm of stripe x [P, 2, hi-lo] ->
            o_tile[:, :, 1+lo:1+hi]."""
            n = hi - lo
            sq = tmp.tile([P, 2, SW], f32, tag="sq")
            for c in range(2):
                nc.scalar.activation(out=sq[:, c, 0:n], in_=x[:, c, 0:n],
                                     func=AF.Square)
            ar0 = ser.tile([P, SW], f32, tag="ar0")
            ar1 = ser.tile([P, SW], f32, tag="ar1")
            aq0 = ser.tile([P, SW], f32, tag="aq0")
            aq1 = ser.tile([P, SW], f32, tag="aq1")
            nc.gpsimd.partition_all_reduce(ar0[:, 0:n], x[:, 0, 0:n], channels=P,
                                           reduce_op=bass_isa.ReduceOp.add)
            nc.gpsimd.partition_all_reduce(ar1[:, 0:n], x[:, 1, 0:n], channels=P,
                                           reduce_op=bass_isa.ReduceOp.add)
            nc.gpsimd.partition_all_reduce(aq0[:, 0:n], sq[:, 0, 0:n], channels=P,
                                           reduce_op=bass_isa.ReduceOp.add)
            nc.gpsimd.partition_all_reduce(aq1[:, 0:n], sq[:, 1, 0:n], channels=P,
                                           reduce_op=bass_isa.ReduceOp.add)
            s1 = ser.tile([P, SW], f32, tag="s1")
            s2 = ser.tile([P, SW], f32, tag="s2")
            nc.vector.tensor_add(out=s1[:, 0:n], in0=ar0[:, 0:n], in1=ar1[:, 0:n])
            nc.vector.tensor_add(out=s2[:, 0:n], in0=aq0[:, 0:n], in1=aq1[:, 0:n])
            t = ser.tile([P, SW], f32, tag="t")
            nc.scalar.activation(out=t[:, 0:n], in_=s1[:, 0:n], func=AF.Square)
            nc.vector.scalar_tensor_tensor(out=t[:, 0:n], in0=t[:, 0:n],
                                           scalar=-1.0 / C, in1=s2[:, 0:n],
                                           op0=ALU.mult, op1=ALU.add)
            nc.scalar.activation(out=t[:, 0:n], in_=t[:, 0:n], func=AF.Sqrt,
                                 scale=1.0 / C, bias=eps[:, :])
            rs = ser.tile([P, SW], f32, tag="rs")
            nc.vector.reciprocal(out=rs[:, 0:n], in_=t[:, 0:n])
            for c in range(2):
                t1 = ser.tile([P, SW], f32, tag="t1")
                nc.vector.scalar_tensor_tensor(out=t1[:, 0:n], in0=s1[:, 0:n],
                                               scalar=-1.0 / C, in1=x[:, c, 0:n],
                                               op0=ALU.mult, op1=ALU.add)
                nc.vector.tensor_mul(out=t1[:, 0:n], in0=t1[:, 0:n], in1=rs[:, 0:n])
                nc.scalar.activation(out=o_tile[:, c, 1 + lo:1 + hi], in_=t1[:, 0:n],
                                     func=AF.Identity,
                                     scale=gb[:, c, 2 * gi:2 * gi + 1],
                                     bias=gb[:, c, 2 * gi + 1:2 * gi + 2])
                if extra_tile is not None:
                    nc.vector.tensor_copy(out=extra_tile[:, c, 1 + lo:1 + hi],
                                          in_=o_tile[:, c, 1 + lo:1 + hi].bitcast(f32))

        def stripes_of(n, edge=0):
            """split [0, n) into even-width stripes; first stripe ends at
            edge (even) if given, else an even half."""
            if n <= SW:
                return [(0, n)]
            h = edge if edge else ((n // 2 + 1) & ~1)
            return [(0, h), (h, n)]

        def mb_stripe(m, j, lo, hi, pj, o_m, dup_m):
            """conv+gate+inject+cln for modality m, stripe [lo, hi)."""
            n = hi - lo
            xin = tmp.tile([P, 2, SW], f32, tag=f"xin{m}", name=f"xin{m}")
            pf = [None, None]
            pg = [None, None]
            for co in range(2):
                pf[co] = ps.tile([P, SW], f32, tag="ps", name=f"pf{co}")
                conv(pf[co], 2 * m + 0, o_prev[m], co, lo, hi, pj)
            for co in range(2):
                pg[co] = ps.tile([P, SW], f32, tag="ps", name=f"pg{co}")
                conv(pg[co], 2 * m + 1, o_prev[m], co, lo, hi, pj)
            tf = [None, None]
            tg = [None, None]
            for co in range(2):
                tf[co] = tmp.tile([P, SW], f32, tag=f"tf{co}", name=f"tf{co}")
                nc.scalar.activation(out=tf[co][:, 0:n], in_=pf[co][:, 0:n],
                                     func=AF.Tanh)
            for co in range(2):
                tg[co] = tmp.tile([P, SW], f32, tag=f"tg{co}", name=f"tg{co}")
                nc.scalar.activation(out=tg[co][:, 0:n], in_=pg[co][:, 0:n],
                                     func=AF.Sigmoid)
            for co in range(2):
                hx = tmp.tile([P, SW], f32, tag="hx")
                nc.vector.tensor_mul(out=hx[:, 0:n], in0=tf[co][:, 0:n],
                                     in1=tg[co][:, 0:n])
                nc.vector.tensor_add(out=xin[:, co, 0:n], in0=hx[:, 0:n],
                                     in1=feats[m][:, co, 1 + lo:1 + hi])
            cln(xin, lo, hi, m, o_m, extra_tile=dup_m)

        o_prev = [None, None]   # o0_{j-1}, o1_{j-1} (as read by mb convs)
        of_prev = None
        for j in [jj for _ in range(repeat) for jj in range(1, A + 1)]:
            w = _w(j)
            u = w + 2      # o-block compute width, rounded up to even (fp32r
                           # matmuls require an even moving free-dim)
            pj = _prec(j)
            pnext = _prec(j + 1) if j < A else 'f'
            dt_o = f32r if pj == 'r' else f32
            dt_of = f32r if pnext == 'r' else f32
            need_dup = (pj == 'r' and pnext == 'f')

            ostr = stripes_of(u)
            h0 = ostr[0][1]
            # fusion stripes end 2 short of the o-stripe boundary so the
            # first fusion stripe depends only on the first o-stripe
            fstr = stripes_of(w, edge=(h0 - 2 if len(ostr) > 1 else 0))

            o_cur = []
            dup_cur = []
            for m in range(2):
                pool_m = po0 if m == 0 else po1
                o_m = pool_m.tile([P, 2, FW + 1], dt_o, tag=f"o{m}", name=f"o_m{m}")
                nc.vector.tensor_copy(out=o_m[:, :, 0:1], in_=zc)
                dup_m = None
                if need_dup:
                    dup_m = pdup.tile([P, 2, FW + 1], f32, tag=f"dup{m}", name=f"dup{m}")
                    nc.vector.tensor_copy(out=dup_m[:, :, 0:1], in_=zc)
                o_cur.append(o_m)
                dup_cur.append(dup_m)

            if j == 1:
                # z_0 = 0: h = 0, o = cln(feat)
                for (lo, hi) in ostr:
                    for m in range(2):
                        xs = tmp.tile([P, 2, SW], f32, tag=f"xin{m}", name=f"x1{m}")
                        for c in range(2):
                            nc.vector.tensor_copy(out=xs[:, c, 0:hi - lo],
                                                  in_=feats[m][:, c, 1 + lo:1 + hi])
                        cln(xs, lo, hi, m, o_cur[m], extra_tile=dup_cur[m])
            else:
                for (lo, hi) in ostr:
                    for m in range(2):
                        mb_stripe(m, j, lo, hi, pj, o_cur[m], dup_cur[m])

            # self conv: of_prev is ready from the previous app; emit before
            # fusion so PE has work while the o-clns drain
            pslf = {}
            if j > 1:
                for (lo, hi) in fstr:
                    for co in range(2):
                        t_ = ps.tile([P, SW], f32, tag="ps", name=f"pslf{co}")
                        conv(t_, 8, of_prev, co, lo, hi, pj)
                        pslf[(lo, co)] = t_

            of_t = pof.tile([P, 2, FW + 1], dt_of, tag="of")
            nc.vector.tensor_copy(out=of_t[:, :, 0:1], in_=zc)

            for (lo, hi) in fstr:
                n = hi - lo
                acc = tmp.tile([P, 2, SW], f32, tag="acc")
                for m in range(2):
                    pgt = [None, None]
                    ppt = [None, None]
                    for co in range(2):
                        pgt[co] = ps.tile([P, SW], f32, tag="ps", name=f"pgt{co}")
                        conv(pgt[co], 4 + 2 * m, o_cur[m], co, lo, hi, pj)
                    for co in range(2):
                        ppt[co] = ps.tile([P, SW], f32, tag="ps", name=f"ppt{co}")
                        conv(ppt[co], 5 + 2 * m, o_cur[m], co, lo, hi, pj)
                    sg = [None, None]
                    for co in range(2):
                        sg[co] = tmp.tile([P, SW], f32, tag=f"sg{co}", name=f"sg{co}")
                        nc.scalar.activation(out=sg[co][:, 0:n], in_=pgt[co][:, 0:n],
                                             func=AF.Sigmoid)
                    for co in range(2):
                        if m == 0:
                            nc.vector.tensor_mul(out=acc[:, co, 0:n],
                                                 in0=sg[co][:, 0:n],
                                                 in1=ppt[co][:, 0:n])
                        else:
                            gp = tmp.tile([P, SW], f32, tag="gp")
                            nc.vector.tensor_mul(out=gp[:, 0:n], in0=sg[co][:, 0:n],
                                                 in1=ppt[co][:, 0:n])
                            nc.vector.tensor_add(out=acc[:, co, 0:n],
                                                 in0=acc[:, co, 0:n],
                                                 in1=gp[:, 0:n])
                xf = tmp.tile([P, 2, SW], f32, tag="xf")
                for co in range(2):
                    if j == 1:
                        nc.vector.tensor_add(out=xf[:, co, 0:n],
                                             in0=acc[:, co, 0:n],
                                             in1=feats[2][:, co, 1 + lo:1 + hi])
                    else:
                        nc.vector.tensor_add(out=xf[:, co, 0:n],
                                             in0=pslf[(lo, co)][:, 0:n],
                                             in1=acc[:, co, 0:n])
                        nc.vector.tensor_add(out=xf[:, co, 0:n], in0=xf[:, co, 0:n],
                                             in1=feats[2][:, co, 1 + lo:1 + hi])
                cln(xf, lo, hi, 2, of_t)

            o_prev = [dup_cur[0] if need_dup else o_cur[0],
                      dup_cur[1] if need_dup else o_cur[1]]
            of_prev = of_t

            if j == A:
                nc.sync.dma_start(out=out_d.ap()[:, 0:2, :],
                                  in_=o_cur[0][:, :, 1:1 + HALF].bitcast(f32))
                nc.sync.dma_start(out=out_d.ap()[:, 2:4, :],
                                  in_=o_cur[1][:, :, 1:1 + HALF].bitcast(f32))
                nc.sync.dma_start(out=out_d.ap()[:, 4:6, :],
                                  in_=of_t[:, :, 1:1 + HALF].bitcast(f32))

    nc.compile()
    return nc


_NC = None


def _get_nc():
    global _NC
    if _NC is None:
        _NC = build_nc()
    return _NC


def _prep_inputs(inputs):
    """Build the 8 per-core input maps from the full problem inputs."""
    feats = [np.asarray(inputs[f"feat{i}"], np.float32) for i in range(3)]
    # stationary weights, one per side (k flipped for the mirrored side)
    wst = []
    for side in range(2):
        arr = np.empty((P, NTILES, P), np.float32)
        for cv, name in enumerate(CONVS):
            Wt = np.asarray(inputs[name], np.float32)   # [co, ci, k]
            if side == 1:
                Wt = Wt[:, :, ::-1]
            # arr[p, tidx(cv,k,ci,co), m] = Wt[co*128+m, ci*128+p, k]
            r = Wt.reshape(2, P, 2, P, 3)                # [coc, m, cic, p, k]
            r = r.transpose(3, 4, 2, 0, 1)               # [p, k, cic, coc, m]
            arr[:, cv * 12:(cv + 1) * 12, :] = r.reshape(P, 12, P)
        wst.append(np.ascontiguousarray(arr))
    gba = np.empty((P, 2, 6), np.float32)
    for gi, (gn, bn) in enumerate([("mb0_gamma", "mb0_beta"),
                                   ("mb1_gamma", "mb1_beta"),
                                   ("fb_gamma", "fb_beta")]):
        gba[:, :, 2 * gi] = np.asarray(inputs[gn], np.float32).reshape(2, P).T
        gba[:, :, 2 * gi + 1] = np.asarray(inputs[bn], np.float32).reshape(2, P).T
    gba = np.ascontiguousarray(gba)

    in_maps = []
    for c in range(8):
        b, side = c // 2, c % 2
        m = {"wst": wst[side], "gb": gba}
        for i in range(3):
            ft = np.zeros((P, 2, FW + 1), np.float32)
            sl = feats[i][b, :, 0:FW] if side == 0 else feats[i][b, :, T - FW:][:, ::-1]
            ft[:, :, 1:] = sl.reshape(2, P, FW).transpose(1, 0, 2)
            m[f"f{i}"] = ft
        in_maps.append(m)
    return in_maps


def run(inputs, **kw):
    nc = _get_nc()
    in_maps = _prep_inputs(inputs)
    res = run_bass_kernel_spmd(nc, in_maps, list(range(8)), **kw)
    out = np.empty((B, 3 * C, T), np.float32)
    for c in range(8):
        b, side = c // 2, c % 2
        o = res.results[c]["out"]            # [P, 6, HALF]
        o = o.transpose(1, 0, 2).reshape(3 * C, HALF)   # rows blk*128+p
        if side == 0:
            out[b, :, 0:HALF] = o
        else:
            out[b, :, HALF:] = o[:, ::-1]
    return out, res


def kernel(**inputs) -> np.ndarray:
    out, _ = run(inputs)
    return out
